# revision 30
# baseline (speedup 1.0000x reference)
"""BiDAF-style co-attention (memory_len=2) Trainium2 Bass kernel.

Full inputs:
  input     [8, 4096, 1024] f32
  memory    [8, 2, 1024]    f32
  w_input   [1024] f32, w_memory [1024] f32, dot_scale [1024] f32
Output:
  concat([input, output_one, input*output_one, output_two*output_one], -1)
  -> [8, 4096, 4096] f32

Sharding: data-parallel over batch; core b gets batch b (8 cores).

The kernel is HBM-bound (16MB in + 64MB out per core at ~358 GB/s).
Everything computational is sized to stay off the critical path:
  - input is cast to bf16 during the SWDGE load (rel tolerance is 2e-2;
    bf16 keeps us ~2e-3) -> DVE dot products run in the 2x perf tier
  - att dots via DVE fused mult-reduce against broadcast v_m = w_input +
    dot_scale*mem_m (bf16)
  - output_one / comp3 numerators via bf16 PE outer products (e0,e1
    stationary), r applied on the PSUM->SBUF ACT copy
  - output_two partials accumulate on the PE (wexp column stationary,
    PSUM accumulate across all 32 tiles) instead of 32 serial DVE ops
  - all broadcasts (v0/v1, cdiff, q vectors) are built on-chip with
    ones/selector matmuls -- no HBM round trips
  - comp0 (input passthrough) is a SWDGE cast store bf16->f32
  - stores are spread across sync/scalar HWDGE + gpsimd SWDGE queues
"""

import numpy as np

B, L, D = 8, 4096, 1024
T = L // 128  # 32 row-tiles of 128
G = 8         # tiles per group (batched small ops)
NG = T // G

_CACHE = {}

# stats column layout ([128, NSTAT] f32), blocks of 32 (col t = tile t)
A0 = 0      # att0
A1 = 32     # att1
AM = 64     # amax
E0 = 96     # e0arg -> e0   (E1 = E0+32 so (e0_t, e1_t) is a stride-32 pair)
E1 = 128    # e1arg -> e1
WE = 160    # wexp = exp(amax)
SS = 192    # e0+e1
RR = 224    # r = 1/(e0+e1)
OC = 256    # ones column
CD = 257    # cdiff broadcast column
SE = 258    # rowsum of wexp
MD = 259    # memdot col ([2,1])
CS = 260    # cdiff scalar (p0)
ST = 261    # sum wexp total (p0)
SR = 262    # 1/ST (p0)
NSTAT = 263


def _build():
    import concourse.bacc as bacc
    import concourse.bass as bass
    import concourse.tile as tile
    from concourse import mybir
    from concourse.masks import make_identity

    f32 = mybir.dt.float32
    bf16 = mybir.dt.bfloat16
    ALU = mybir.AluOpType
    ACT = mybir.ActivationFunctionType

    nc = bacc.Bacc("TRN2", target_bir_lowering=False, debug=False)

    inp = nc.dram_tensor("input", [L, D], f32, kind="ExternalInput").ap()
    mem = nc.dram_tensor("memory", [2, D], f32, kind="ExternalInput").ap()
    w_in = nc.dram_tensor("w_input", [D], f32, kind="ExternalInput").ap()
    w_mem = nc.dram_tensor("w_memory", [D], f32, kind="ExternalInput").ap()
    d_sc = nc.dram_tensor("dot_scale", [D], f32, kind="ExternalInput").ap()
    out = nc.dram_tensor("out", [L, 4 * D], f32, kind="ExternalOutput").ap()

    def bc(src_ap, n_part, n_free):
        # broadcast-read AP: n_part partitions each reading the same n_free
        # contiguous elements at src_ap's offset (DMA-only pattern)
        return bass.AP(src_ap.tensor, src_ap.offset, [[0, n_part], [1, n_free]])

    ts = bass.ts

    with tile.TileContext(nc) as tc:
        with (
            tc.tile_pool(name="consts", bufs=1) as consts,
            tc.tile_pool(name="setup2d", bufs=4) as setup2d,
            tc.tile_pool(name="inp_pool", bufs=32) as inp_pool,
            tc.tile_pool(name="scratch", bufs=4) as scratch,
            tc.tile_pool(name="stage12", bufs=6) as stage12p,
            tc.tile_pool(name="stage3", bufs=6) as stage3p,
            tc.tile_pool(name="o1ps", bufs=2, space="PSUM") as o1psp,
            tc.tile_pool(name="wstps", bufs=2, space="PSUM") as wstpsp,
            tc.tile_pool(name="o2ps", bufs=1, space="PSUM") as o2psp,
        ):
            # ---------------- input prefetch ----------------
            # group-0 cast loads go first so SWDGE starts moving bytes at
            # t=0 (identity/memset setup also lives on gpsimd)
            in_ts = {}

            def issue_loads(g):
                for t in range(g, min(g + G, T)):
                    in_t = inp_pool.tile([128, D], bf16, tag="in_t")
                    in_ts[t] = in_t
                    nc.gpsimd.dma_start(out=in_t, in_=inp[ts(t, 128), :])

            issue_loads(0)

            # ---------------- setup ----------------
            mem_sb = consts.tile([2, D], f32)
            nc.sync.dma_start(out=mem_sb, in_=mem)
            mem_bf = consts.tile([2, D], bf16)
            nc.scalar.copy(mem_bf, mem_sb)
            stats = consts.tile([128, NSTAT], f32)
            identity = consts.tile([128, 128], f32)
            make_identity(nc, identity)
            nc.vector.memset(stats[:, OC : OC + 1], 1.0)
            # est: per-tile transposed [e0;e1] stationaries, bf16, col-block t
            est = consts.tile([2, T * 128], bf16)
            web_bf = consts.tile([128, T], bf16)
            # selector / ones stationaries for on-chip broadcasts
            ones_r = consts.tile([1, 128], f32)
            nc.vector.memset(ones_r, 1.0)
            ones_bf = consts.tile([1, 128], bf16)
            nc.vector.memset(ones_bf, 1.0)
            sel = consts.tile([2, 256], bf16)
            nc.vector.memset(sel, 0.0)
            nc.vector.memset(sel[0:1, 0:128], 1.0)
            # row-1 writes need a DMA (engine APs must be partition-aligned)
            nc.scalar.dma_start(out=sel[1:2, 128:256], in_=ones_bf)
            pm = consts.tile([2, 1], f32)
            nc.vector.memset(pm, 1.0)
            nc.vector.memset(pm[0:1, :], -1.0)
            # strided pair view: pair_view[:, t, :] = cols (E0+t, E1+t)
            pair_view = stats[:, E0 : E0 + 64].rearrange("p (a b) -> p b a", a=2)

            ds_b = setup2d.tile([2, D], f32, tag="s2d")
            nc.scalar.dma_start(out=ds_b, in_=bc(d_sc, 2, D))
            win_b = setup2d.tile([2, D], f32, tag="s2d")
            nc.scalar.dma_start(out=win_b, in_=bc(w_in, 2, D))
            # v_cat = mem*ds + w_in  (rows: v0, v1)
            vcat = setup2d.tile([2, D], f32, tag="s2d")
            nc.vector.tensor_tensor(out=vcat, in0=mem_sb, in1=ds_b, op=ALU.mult)
            nc.vector.tensor_tensor(out=vcat, in0=vcat, in1=win_b, op=ALU.add)
            vcat_bf = setup2d.tile([2, D], bf16, tag="vbf")
            nc.scalar.copy(vcat_bf, vcat)
            # broadcast v0/v1 to 128 partitions via selector matmuls (bf16)
            v0b = consts.tile([128, D], bf16)
            v1b = consts.tile([128, D], bf16)
            for m, dst in ((0, v0b), (1, v1b)):
                vps = o1psp.tile([128, D], f32, tag="o1")
                for h in range(2):
                    nc.tensor.matmul(
                        vps[:, ts(h, 512)],
                        lhsT=sel[:, ts(m, 128)],
                        rhs=vcat_bf[:, ts(h, 512)],
                        start=True,
                        stop=True,
                    )
                nc.scalar.copy(dst, vps)

            # memdot = (mem * w_memory).sum(-1) -> [2,1]; cdiff = c1-c0
            wmem_b = setup2d.tile([2, D], f32, tag="s2d")
            nc.sync.dma_start(out=wmem_b, in_=bc(w_mem, 2, D))
            sc2 = setup2d.tile([2, D], f32, tag="s2d")
            nc.vector.scalar_tensor_tensor(
                out=sc2, in0=mem_sb, scalar=1.0, in1=wmem_b,
                op0=ALU.mult, op1=ALU.mult,
                accum_out=stats[0:2, MD : MD + 1],
            )
            # cdiff scalar on p0 via [-1,+1] matmul, then broadcast to col
            cd_ps = wstpsp.tile([1, 1], f32, tag="wst")
            nc.tensor.matmul(
                cd_ps, lhsT=pm, rhs=stats[0:2, MD : MD + 1], start=True, stop=True
            )
            nc.scalar.copy(stats[0:1, CS : CS + 1], cd_ps)
            cdb_ps = wstpsp.tile([128, 1], f32, tag="wst")
            nc.tensor.matmul(
                cdb_ps, lhsT=ones_r, rhs=stats[0:1, CS : CS + 1],
                start=True, stop=True,
            )
            nc.scalar.copy(stats[:, CD : CD + 1], cdb_ps)
            cdc = stats[:, CD : CD + 1]

            # output_two partials accumulate here across the whole pass
            o2s_ps = o2psp.tile([2, D], f32, tag="o2")

            # ---------------- main pass ----------------
            for g in range(0, T, G):
                for t in range(g, g + G):
                    in_t = in_ts[t]
                    # next-group load prefetch first (never queued behind
                    # comp0's load-complete wait), then comp0 -- issued
                    # early, it decouples 25% of the output bytes from the
                    # compute chain
                    if t + G < T:
                        tn = t + G
                        in_n = inp_pool.tile([128, D], bf16, tag="in_t")
                        in_ts[tn] = in_n
                        nc.gpsimd.dma_start(out=in_n, in_=inp[ts(tn, 128), :])
                    nc.gpsimd.dma_start(out=out[ts(t, 128), 0:D], in_=in_t)
                    # two fused att dots (DVE, bf16 2x tier)
                    sc_t = scratch.tile([128, D], bf16, tag="ttr")
                    nc.vector.scalar_tensor_tensor(
                        out=sc_t, in0=in_t, scalar=1.0, in1=v0b,
                        op0=ALU.mult, op1=ALU.mult,
                        accum_out=stats[:, A0 + t : A0 + t + 1],
                    )
                    sc_t2 = scratch.tile([128, D], bf16, tag="ttr")
                    nc.vector.scalar_tensor_tensor(
                        out=sc_t2, in0=in_t, scalar=1.0, in1=v1b,
                        op0=ALU.mult, op1=ALU.mult,
                        accum_out=stats[:, A1 + t : A1 + t + 1],
                    )

                # batched group stats ([128, G] blocks)
                a0b = stats[:, A0 + g : A0 + g + G]
                a1b = stats[:, A1 + g : A1 + g + G]
                amb = stats[:, AM + g : AM + g + G]
                e0b = stats[:, E0 + g : E0 + g + G]
                e1b = stats[:, E1 + g : E1 + g + G]
                web = stats[:, WE + g : WE + g + G]
                ssb = stats[:, SS + g : SS + g + G]
                rrb = stats[:, RR + g : RR + g + G]
                # amax = max(a1 + cdiff, a0)
                nc.vector.scalar_tensor_tensor(
                    out=amb, in0=a1b, scalar=cdc, in1=a0b,
                    op0=ALU.add, op1=ALU.max,
                )
                # e0arg = a0 - amax ; e1arg = (a1 + cdiff) - amax
                nc.vector.tensor_tensor(out=e0b, in0=a0b, in1=amb, op=ALU.subtract)
                nc.vector.scalar_tensor_tensor(
                    out=e1b, in0=a1b, scalar=cdc, in1=amb,
                    op0=ALU.add, op1=ALU.subtract,
                )
                nc.scalar.activation(out=e0b, in_=e0b, func=ACT.Exp)
                nc.scalar.activation(out=e1b, in_=e1b, func=ACT.Exp)
                nc.scalar.activation(out=web, in_=amb, func=ACT.Exp)
                nc.vector.tensor_tensor(out=ssb, in0=e0b, in1=e1b, op=ALU.add)
                nc.vector.reciprocal(rrb, ssb)
                nc.scalar.copy(web_bf[:, g : g + G], web)

                for t in range(g, g + G):
                    wst_ps = wstpsp.tile([2, 128], f32, tag="wst")
                    nc.tensor.transpose(wst_ps, pair_view[:, t, :], identity)
                    nc.scalar.copy(est[:, ts(t, 128)], wst_ps)

                for t in range(g, g + G):
                    in_t = in_ts[t]
                    rc = stats[:, RR + t : RR + t + 1]
                    lhs_t = est[:, ts(t, 128)]
                    # output_one numerator = e0*mem0 + e1*mem1 (bf16 PE)
                    o1_ps = o1psp.tile([128, D], f32, tag="o1")
                    for h in range(2):
                        nc.tensor.matmul(
                            o1_ps[:, ts(h, 512)],
                            lhsT=lhs_t,
                            rhs=mem_bf[:, ts(h, 512)],
                            start=True,
                            stop=True,
                        )
                    st12 = stage12p.tile([128, 2 * D], f32, tag="s12")
                    # normalized output_one on the PSUM->SBUF copy
                    nc.scalar.activation(
                        out=st12[:, 0:D], in_=o1_ps, func=ACT.Copy, scale=rc
                    )
                    # comp2 = input * output_one = (input*r) * o1_num
                    nc.vector.scalar_tensor_tensor(
                        out=st12[:, D : 2 * D], in0=in_t, scalar=rc,
                        in1=o1_ps, op0=ALU.mult, op1=ALU.mult,
                    )
                    st_eng = nc.sync if t % 2 == 0 else nc.scalar
                    st_eng.dma_start(out=out[ts(t, 128), D : 3 * D], in_=st12)
                    # output_two partials: PSUM-accumulating PE matvec
                    for h in range(2):
                        nc.tensor.matmul(
                            o2s_ps[0:1, ts(h, 512)],
                            lhsT=web_bf[:, t : t + 1],
                            rhs=in_t[:, ts(h, 512)],
                            start=(t == 0),
                            stop=(t == T - 1),
                            skip_group_check=True,
                        )

            # ---------------- output_two normalize + q vectors ----------------
            nc.vector.tensor_reduce(
                out=stats[:, SE : SE + 1], in_=stats[:, WE : WE + T],
                axis=mybir.AxisListType.X, op=ALU.add,
            )
            stot_ps = wstpsp.tile([1, 1], f32, tag="wst")
            nc.tensor.matmul(
                stot_ps, lhsT=stats[:, SE : SE + 1], rhs=stats[:, OC : OC + 1],
                start=True, stop=True,
            )
            nc.scalar.copy(stats[0:1, ST : ST + 1], stot_ps)
            nc.vector.reciprocal(stats[0:1, SR : SR + 1], stats[0:1, ST : ST + 1])
            o2n = setup2d.tile([1, D], f32, tag="s2d")
            nc.scalar.activation(
                out=o2n, in_=o2s_ps[0:1, :], func=ACT.Copy,
                scale=stats[0:1, SR : SR + 1],
            )
            # q_m = output_two * mem_m : broadcast o2n to 2 partitions (PE),
            # multiply by mem on the DVE, cast to bf16
            q_ps = o2psp.tile([2, D], f32, tag="o2")
            for h in range(2):
                nc.tensor.matmul(
                    q_ps[:, ts(h, 512)],
                    lhsT=ones_r[:, 0:2],
                    rhs=o2n[:, ts(h, 512)],
                    start=True,
                    stop=True,
                )
            qcat_bf = setup2d.tile([2, D], bf16, tag="qbf")
            nc.vector.tensor_tensor(out=qcat_bf, in0=q_ps, in1=mem_sb, op=ALU.mult)

            # ---------------- comp3 pass (PE + ACT) ---------------------------
            for t in range(T):
                rc = stats[:, RR + t : RR + t + 1]
                lhs_t = est[:, ts(t, 128)]
                ps3 = o1psp.tile([128, D], f32, tag="o1")
                for h in range(2):
                    nc.tensor.matmul(
                        ps3[:, ts(h, 512)],
                        lhsT=lhs_t,
                        rhs=qcat_bf[:, ts(h, 512)],
                        start=True,
                        stop=True,
                    )
                st3 = stage3p.tile([128, D], f32, tag="s3")
                nc.scalar.activation(out=st3, in_=ps3, func=ACT.Copy, scale=rc)
                eng = (nc.sync, nc.scalar, nc.gpsimd)[t % 3]
                eng.dma_start(out=out[ts(t, 128), 3 * D : 4 * D], in_=st3)

    nc.compile()
    return nc


def _get_nc():
    if "nc" not in _CACHE:
        _CACHE["nc"] = _build()
    return _CACHE["nc"]


def kernel(input, memory, w_input, w_memory, dot_scale):
    from concourse.bass_utils import run_bass_kernel_spmd

    nc = _get_nc()
    input = np.ascontiguousarray(input, dtype=np.float32)
    memory = np.ascontiguousarray(memory, dtype=np.float32)
    w_input = np.ascontiguousarray(w_input, dtype=np.float32)
    w_memory = np.ascontiguousarray(w_memory, dtype=np.float32)
    dot_scale = np.ascontiguousarray(dot_scale, dtype=np.float32)
    in_maps = [
        {
            "input": input[b],
            "memory": memory[b],
            "w_input": w_input,
            "w_memory": w_memory,
            "dot_scale": dot_scale,
        }
        for b in range(B)
    ]
    res = run_bass_kernel_spmd(nc, in_maps, core_ids=list(range(B)))
    return np.stack([res.results[b]["out"] for b in range(B)], axis=0)


# revision 32
# speedup vs baseline: 1.0232x; 1.0232x over previous
"""BiDAF-style co-attention (memory_len=2) Trainium2 Bass kernel.

Full inputs:
  input     [8, 4096, 1024] f32
  memory    [8, 2, 1024]    f32
  w_input   [1024] f32, w_memory [1024] f32, dot_scale [1024] f32
Output:
  concat([input, output_one, input*output_one, output_two*output_one], -1)
  -> [8, 4096, 4096] f32

Sharding: data-parallel over batch; core b gets batch b (8 cores).

The kernel is HBM-bound (16MB in + 64MB out per core at ~358 GB/s).
Everything computational is sized to stay off the critical path:
  - input is cast to bf16 during the SWDGE load (rel tolerance is 2e-2;
    bf16 keeps us ~2e-3) -> DVE dot products run in the 2x perf tier
  - att dots via DVE fused mult-reduce against broadcast v_m = w_input +
    dot_scale*mem_m (bf16)
  - output_one / comp3 numerators via bf16 PE outer products (e0,e1
    stationary), r applied on the PSUM->SBUF ACT copy
  - output_two partials accumulate on the PE (wexp column stationary,
    PSUM accumulate across all 32 tiles) instead of 32 serial DVE ops
  - all broadcasts (v0/v1, cdiff, q vectors) are built on-chip with
    ones/selector matmuls -- no HBM round trips
  - comp0 (input passthrough) is a SWDGE cast store bf16->f32
  - stores are spread across sync/scalar HWDGE + gpsimd SWDGE queues
"""

import numpy as np

B, L, D = 8, 4096, 1024
T = L // 128  # 32 row-tiles of 128
G = 8         # tiles per group (batched small ops)
NG = T // G

_CACHE = {}

# stats column layout ([128, NSTAT] f32), blocks of 32 (col t = tile t)
A0 = 0      # att0
A1 = 32     # att1
AM = 64     # amax
E0 = 96     # e0arg -> e0   (E1 = E0+32 so (e0_t, e1_t) is a stride-32 pair)
E1 = 128    # e1arg -> e1
WE = 160    # wexp = exp(amax)
SS = 192    # e0+e1
RR = 224    # r = 1/(e0+e1)
OC = 256    # ones column
CD = 257    # cdiff broadcast column
SE = 258    # rowsum of wexp
MD = 259    # memdot col ([2,1])
CS = 260    # cdiff scalar (p0)
ST = 261    # sum wexp total (p0)
SR = 262    # 1/ST (p0)
NSTAT = 263


def _build():
    import concourse.bacc as bacc
    import concourse.bass as bass
    import concourse.tile as tile
    from concourse import mybir
    from concourse.masks import make_identity

    f32 = mybir.dt.float32
    bf16 = mybir.dt.bfloat16
    ALU = mybir.AluOpType
    ACT = mybir.ActivationFunctionType

    nc = bacc.Bacc("TRN2", target_bir_lowering=False, debug=False)

    inp = nc.dram_tensor("input", [L, D], f32, kind="ExternalInput").ap()
    mem = nc.dram_tensor("memory", [2, D], f32, kind="ExternalInput").ap()
    w_in = nc.dram_tensor("w_input", [D], f32, kind="ExternalInput").ap()
    w_mem = nc.dram_tensor("w_memory", [D], f32, kind="ExternalInput").ap()
    d_sc = nc.dram_tensor("dot_scale", [D], f32, kind="ExternalInput").ap()
    out = nc.dram_tensor("out", [L, 4 * D], f32, kind="ExternalOutput").ap()

    def bc(src_ap, n_part, n_free):
        # broadcast-read AP: n_part partitions each reading the same n_free
        # contiguous elements at src_ap's offset (DMA-only pattern)
        return bass.AP(src_ap.tensor, src_ap.offset, [[0, n_part], [1, n_free]])

    ts = bass.ts

    with tile.TileContext(nc) as tc:
        with (
            tc.tile_pool(name="consts", bufs=1) as consts,
            tc.tile_pool(name="setup2d", bufs=4) as setup2d,
            tc.tile_pool(name="inp_pool", bufs=32) as inp_pool,
            tc.tile_pool(name="scratch", bufs=4) as scratch,
            tc.tile_pool(name="stage12", bufs=6) as stage12p,
            tc.tile_pool(name="stage3", bufs=6) as stage3p,
            tc.tile_pool(name="o1ps", bufs=2, space="PSUM") as o1psp,
            tc.tile_pool(name="wstps", bufs=2, space="PSUM") as wstpsp,
            tc.tile_pool(name="o2ps", bufs=1, space="PSUM") as o2psp,
        ):
            # ---------------- input prefetch ----------------
            # group-0 cast loads go first so SWDGE starts moving bytes at
            # t=0 (identity/memset setup also lives on gpsimd)
            in_ts = {}

            def issue_loads(g):
                for t in range(g, min(g + G, T)):
                    in_t = inp_pool.tile([128, D], bf16, tag="in_t")
                    in_ts[t] = in_t
                    nc.gpsimd.dma_start(out=in_t, in_=inp[ts(t, 128), :])

            issue_loads(0)

            # ---------------- setup ----------------
            mem_sb = consts.tile([2, D], f32)
            nc.sync.dma_start(out=mem_sb, in_=mem)
            mem_bf = consts.tile([2, D], bf16)
            nc.scalar.copy(mem_bf, mem_sb)
            stats = consts.tile([128, NSTAT], f32)
            identity = consts.tile([128, 128], f32)
            make_identity(nc, identity)
            nc.vector.memset(stats[:, OC : OC + 1], 1.0)
            # est: per-tile transposed [e0;e1] stationaries, bf16, col-block t
            est = consts.tile([2, T * 128], bf16)
            web_bf = consts.tile([128, T], bf16)
            # selector / ones stationaries for on-chip broadcasts
            ones_r = consts.tile([1, 128], f32)
            nc.vector.memset(ones_r, 1.0)
            ones_bf = consts.tile([1, 128], bf16)
            nc.vector.memset(ones_bf, 1.0)
            sel = consts.tile([2, 256], bf16)
            nc.vector.memset(sel, 0.0)
            nc.vector.memset(sel[0:1, 0:128], 1.0)
            # row-1 writes need a DMA (engine APs must be partition-aligned)
            nc.scalar.dma_start(out=sel[1:2, 128:256], in_=ones_bf)
            pm = consts.tile([2, 1], f32)
            nc.vector.memset(pm, 1.0)
            nc.vector.memset(pm[0:1, :], -1.0)
            # strided pair view: pair_view[:, t, :] = cols (E0+t, E1+t)
            pair_view = stats[:, E0 : E0 + 64].rearrange("p (a b) -> p b a", a=2)

            ds_b = setup2d.tile([2, D], f32, tag="s2d")
            nc.scalar.dma_start(out=ds_b, in_=bc(d_sc, 2, D))
            win_b = setup2d.tile([2, D], f32, tag="s2d")
            nc.scalar.dma_start(out=win_b, in_=bc(w_in, 2, D))
            # v_cat = mem*ds + w_in  (rows: v0, v1)
            vcat = setup2d.tile([2, D], f32, tag="s2d")
            nc.vector.tensor_tensor(out=vcat, in0=mem_sb, in1=ds_b, op=ALU.mult)
            nc.vector.tensor_tensor(out=vcat, in0=vcat, in1=win_b, op=ALU.add)
            vcat_bf = setup2d.tile([2, D], bf16, tag="vbf")
            nc.scalar.copy(vcat_bf, vcat)
            # broadcast v0/v1 to 128 partitions via selector matmuls (bf16)
            v0b = consts.tile([128, D], bf16)
            v1b = consts.tile([128, D], bf16)
            for m, dst in ((0, v0b), (1, v1b)):
                vps = o1psp.tile([128, D], f32, tag="o1")
                for h in range(2):
                    nc.tensor.matmul(
                        vps[:, ts(h, 512)],
                        lhsT=sel[:, ts(m, 128)],
                        rhs=vcat_bf[:, ts(h, 512)],
                        start=True,
                        stop=True,
                    )
                nc.scalar.copy(dst, vps)

            # memdot = (mem * w_memory).sum(-1) -> [2,1]; cdiff = c1-c0
            wmem_b = setup2d.tile([2, D], f32, tag="s2d")
            nc.sync.dma_start(out=wmem_b, in_=bc(w_mem, 2, D))
            sc2 = setup2d.tile([2, D], f32, tag="s2d")
            nc.vector.scalar_tensor_tensor(
                out=sc2, in0=mem_sb, scalar=1.0, in1=wmem_b,
                op0=ALU.mult, op1=ALU.mult,
                accum_out=stats[0:2, MD : MD + 1],
            )
            # cdiff scalar on p0 via [-1,+1] matmul, then broadcast to col
            cd_ps = wstpsp.tile([1, 1], f32, tag="wst")
            nc.tensor.matmul(
                cd_ps, lhsT=pm, rhs=stats[0:2, MD : MD + 1], start=True, stop=True
            )
            nc.scalar.copy(stats[0:1, CS : CS + 1], cd_ps)
            cdb_ps = wstpsp.tile([128, 1], f32, tag="wst")
            nc.tensor.matmul(
                cdb_ps, lhsT=ones_r, rhs=stats[0:1, CS : CS + 1],
                start=True, stop=True,
            )
            nc.scalar.copy(stats[:, CD : CD + 1], cdb_ps)
            cdc = stats[:, CD : CD + 1]

            # output_two partials accumulate here across the whole pass
            o2s_ps = o2psp.tile([2, D], f32, tag="o2")

            # ---------------- main pass ----------------
            for g in range(0, T, G):
                for t in range(g, g + G):
                    in_t = in_ts[t]
                    # next-group load prefetch first (never queued behind
                    # comp0's load-complete wait), then comp0 -- issued
                    # early, it decouples 25% of the output bytes from the
                    # compute chain
                    if t + G < T:
                        tn = t + G
                        in_n = inp_pool.tile([128, D], bf16, tag="in_t")
                        in_ts[tn] = in_n
                        nc.gpsimd.dma_start(out=in_n, in_=inp[ts(tn, 128), :])
                    nc.gpsimd.dma_start(out=out[ts(t, 128), 0:D], in_=in_t)
                    # two fused att dots (DVE, bf16 2x tier)
                    sc_t = scratch.tile([128, D], bf16, tag="ttr")
                    nc.vector.scalar_tensor_tensor(
                        out=sc_t, in0=in_t, scalar=1.0, in1=v0b,
                        op0=ALU.mult, op1=ALU.mult,
                        accum_out=stats[:, A0 + t : A0 + t + 1],
                    )
                    sc_t2 = scratch.tile([128, D], bf16, tag="ttr")
                    nc.vector.scalar_tensor_tensor(
                        out=sc_t2, in0=in_t, scalar=1.0, in1=v1b,
                        op0=ALU.mult, op1=ALU.mult,
                        accum_out=stats[:, A1 + t : A1 + t + 1],
                    )

                # batched group stats ([128, G] blocks)
                a0b = stats[:, A0 + g : A0 + g + G]
                a1b = stats[:, A1 + g : A1 + g + G]
                amb = stats[:, AM + g : AM + g + G]
                e0b = stats[:, E0 + g : E0 + g + G]
                e1b = stats[:, E1 + g : E1 + g + G]
                web = stats[:, WE + g : WE + g + G]
                ssb = stats[:, SS + g : SS + g + G]
                rrb = stats[:, RR + g : RR + g + G]
                # amax = max(a1 + cdiff, a0)
                nc.vector.scalar_tensor_tensor(
                    out=amb, in0=a1b, scalar=cdc, in1=a0b,
                    op0=ALU.add, op1=ALU.max,
                )
                # e0arg = a0 - amax ; e1arg = (a1 + cdiff) - amax
                nc.vector.tensor_tensor(out=e0b, in0=a0b, in1=amb, op=ALU.subtract)
                nc.vector.scalar_tensor_tensor(
                    out=e1b, in0=a1b, scalar=cdc, in1=amb,
                    op0=ALU.add, op1=ALU.subtract,
                )
                nc.scalar.activation(out=e0b, in_=e0b, func=ACT.Exp)
                nc.scalar.activation(out=e1b, in_=e1b, func=ACT.Exp)
                nc.scalar.activation(out=web, in_=amb, func=ACT.Exp)
                nc.vector.tensor_tensor(out=ssb, in0=e0b, in1=e1b, op=ALU.add)
                nc.vector.reciprocal(rrb, ssb)
                nc.scalar.copy(web_bf[:, g : g + G], web)

                for t in range(g, g + G):
                    wst_ps = wstpsp.tile([2, 128], f32, tag="wst")
                    nc.tensor.transpose(wst_ps, pair_view[:, t, :], identity)
                    nc.scalar.copy(est[:, ts(t, 128)], wst_ps)

                for t in range(g, g + G):
                    in_t = in_ts[t]
                    rc = stats[:, RR + t : RR + t + 1]
                    lhs_t = est[:, ts(t, 128)]
                    # output_one numerator = e0*mem0 + e1*mem1 (bf16 PE)
                    o1_ps = o1psp.tile([128, D], f32, tag="o1")
                    for h in range(2):
                        nc.tensor.matmul(
                            o1_ps[:, ts(h, 512)],
                            lhsT=lhs_t,
                            rhs=mem_bf[:, ts(h, 512)],
                            start=True,
                            stop=True,
                        )
                    st12 = stage12p.tile([128, 2 * D], f32, tag="s12")
                    # normalized output_one on the PSUM->SBUF copy
                    nc.scalar.activation(
                        out=st12[:, 0:D], in_=o1_ps, func=ACT.Copy, scale=rc
                    )
                    # comp2 = input * output_one = (input*r) * o1_num
                    nc.vector.scalar_tensor_tensor(
                        out=st12[:, D : 2 * D], in0=in_t, scalar=rc,
                        in1=o1_ps, op0=ALU.mult, op1=ALU.mult,
                    )
                    st_eng = nc.sync if t % 2 == 0 else nc.scalar
                    st_eng.dma_start(out=out[ts(t, 128), D : 3 * D], in_=st12)
                    # output_two partials: PSUM-accumulating PE matvec
                    for h in range(2):
                        nc.tensor.matmul(
                            o2s_ps[0:1, ts(h, 512)],
                            lhsT=web_bf[:, t : t + 1],
                            rhs=in_t[:, ts(h, 512)],
                            start=(t == 0),
                            stop=(t == T - 1),
                            skip_group_check=True,
                        )

            # ---------------- output_two normalize + q vectors ----------------
            nc.vector.tensor_reduce(
                out=stats[:, SE : SE + 1], in_=stats[:, WE : WE + T],
                axis=mybir.AxisListType.X, op=ALU.add,
            )
            stot_ps = wstpsp.tile([1, 1], f32, tag="wst")
            nc.tensor.matmul(
                stot_ps, lhsT=stats[:, SE : SE + 1], rhs=stats[:, OC : OC + 1],
                start=True, stop=True,
            )
            nc.scalar.copy(stats[0:1, ST : ST + 1], stot_ps)
            nc.vector.reciprocal(stats[0:1, SR : SR + 1], stats[0:1, ST : ST + 1])
            o2n = setup2d.tile([1, D], bf16, tag="s2d")
            nc.scalar.activation(
                out=o2n, in_=o2s_ps[0:1, :], func=ACT.Copy,
                scale=stats[0:1, SR : SR + 1],
            )
            # q_m = output_two * mem_m : broadcast o2n to 2 partitions (PE,
            # bf16 1-pass), multiply by mem on the DVE, cast to bf16
            q_ps = o2psp.tile([2, D], f32, tag="o2")
            for h in range(2):
                nc.tensor.matmul(
                    q_ps[:, ts(h, 512)],
                    lhsT=ones_bf[:, 0:2],
                    rhs=o2n[:, ts(h, 512)],
                    start=True,
                    stop=True,
                )
            qcat_bf = setup2d.tile([2, D], bf16, tag="qbf")
            nc.vector.tensor_tensor(out=qcat_bf, in0=q_ps, in1=mem_sb, op=ALU.mult)

            # ---------------- comp3 pass (PE + ACT) ---------------------------
            for t in range(T):
                rc = stats[:, RR + t : RR + t + 1]
                lhs_t = est[:, ts(t, 128)]
                ps3 = o1psp.tile([128, D], f32, tag="o1")
                for h in range(2):
                    nc.tensor.matmul(
                        ps3[:, ts(h, 512)],
                        lhsT=lhs_t,
                        rhs=qcat_bf[:, ts(h, 512)],
                        start=True,
                        stop=True,
                    )
                st3 = stage3p.tile([128, D], f32, tag="s3")
                nc.scalar.activation(out=st3, in_=ps3, func=ACT.Copy, scale=rc)
                # stores on sync/gpsimd only -- keeping DMA issues out of
                # the ACT queue, which paces the tail with its st3 copies
                eng = (nc.sync, nc.gpsimd)[t % 2]
                eng.dma_start(out=out[ts(t, 128), 3 * D : 4 * D], in_=st3)

    nc.compile()
    return nc


def _get_nc():
    if "nc" not in _CACHE:
        _CACHE["nc"] = _build()
    return _CACHE["nc"]


def kernel(input, memory, w_input, w_memory, dot_scale):
    from concourse.bass_utils import run_bass_kernel_spmd

    nc = _get_nc()
    input = np.ascontiguousarray(input, dtype=np.float32)
    memory = np.ascontiguousarray(memory, dtype=np.float32)
    w_input = np.ascontiguousarray(w_input, dtype=np.float32)
    w_memory = np.ascontiguousarray(w_memory, dtype=np.float32)
    dot_scale = np.ascontiguousarray(dot_scale, dtype=np.float32)
    in_maps = [
        {
            "input": input[b],
            "memory": memory[b],
            "w_input": w_input,
            "w_memory": w_memory,
            "dot_scale": dot_scale,
        }
        for b in range(B)
    ]
    res = run_bass_kernel_spmd(nc, in_maps, core_ids=list(range(B)))
    return np.stack([res.results[b]["out"] for b in range(B)], axis=0)
